# revision 4
# baseline (speedup 1.0000x reference)
"""Trainium2 Bass kernel for nn_Attention_71846212928150.

Self-attention block (pre-LN + silu, QKV projections, per-head attention with
q/k LayerNorms, output projection), sharded over 8 NeuronCores by heads:
core c owns heads {2c, 2c+1} = inner columns [128c, 128c+128).

v2 design (vs. the fp32r v1): all PE operands are bf16 (PSUM accumulation
stays fp32), all transposes run on the DMA XBAR (dma_start_transpose) instead
of the PE, q/k LN sums ride the QKV matmul as two host-precomputed row-sum
weight columns, the stats AllReduce is split into two chunks overlapped with
compute, the attention loop is software-pipelined (S(kb+1) issued before
PV(kb)) with double-buffered PSUM so the PE never idles, and the softmax
denominators are batched into a [128, 32] reciprocal instead of a 1-partition
15us DVE reciprocal per (batch, head).
"""

import numpy as np

import concourse.bass as bass
import concourse.mybir as mybir
import concourse.tile as tile

F32 = mybir.dt.float32
BF16 = mybir.dt.bfloat16
I32 = mybir.dt.int32
AF = mybir.ActivationFunctionType
ALU = mybir.AluOpType
AX = mybir.AxisListType

B = 2
C = 1024
H = 16
DH = 64
INNER = H * DH
NCORES = 8
HL = H // NCORES          # 2 heads per core
CL = HL * DH              # 128 local inner columns
QKV = 3 * CL              # 384
QKVW = QKV + 2            # + sum_q / sum_k stat columns
KT = C // 128             # 8 contraction tiles over C
EPS = 1e-5
MAGIC = 0x5F3759DF


def _quake_rsqrt(nc, pool, vpe, shape, iters=3, suffix=""):
    """rstd = 1/sqrt(vpe) entirely on DVE (fp32 bitcast + Newton steps)."""
    y = pool.tile(list(shape), F32, name=f"qk_y{suffix}")
    t2 = pool.tile(list(shape), F32, name=f"qk_t2{suffix}")
    nc.vector.tensor_scalar(
        out=y.bitcast(I32), in0=vpe.bitcast(I32), scalar1=1, scalar2=None,
        op0=ALU.logical_shift_right)
    nc.vector.tensor_scalar(
        out=y.bitcast(I32), in0=y.bitcast(I32), scalar1=-1, scalar2=MAGIC,
        op0=ALU.mult, op1=ALU.add)
    for _ in range(iters):
        nc.vector.tensor_tensor(out=t2, in0=y, in1=y, op=ALU.mult)
        nc.vector.tensor_tensor(out=t2, in0=t2, in1=vpe, op=ALU.mult)
        nc.vector.tensor_scalar(out=t2, in0=t2, scalar1=-0.5, scalar2=1.5,
                                op0=ALU.mult, op1=ALU.add)
        nc.vector.tensor_tensor(out=y, in0=y, in1=t2, op=ALU.mult)
    return y


def _fixup_module(nc):
    """Adapt Tile-emitted BIR to this container's walrus build.

    1. The tail `EVENT_SEMAPHORE_RANGE_CLEAR` InstISA (opcode 176) is not
       understood by this walrus' birverifier. Replace it with one
       EventSemaphore sem-write-0 per semaphore in the cleared range.
    2. Drain instructions carrying more than one semaphore wait fail codegen;
       hoist the extra waits into standalone EventSemaphore waits.
    """
    for f in nc.m.functions:
        for bb in f.blocks:
            newlist = []
            changed = False
            for ins in bb.instructions:
                tn = type(ins).__name__
                if tn == "InstISA" and getattr(ins, "isa_opcode", None) == 176:
                    ad = ins.ant_dict or {}
                    first = ad.get("range_first")
                    last = ad.get("range_last")
                    if first is not None and last is not None:
                        si = ins.sync_info
                        sems = list(range(first, last + 1))
                        for k, sem in enumerate(sems):
                            ev = mybir.InstEventSemaphore(
                                name=f"{ins.name}-clr{k}", engine=ins.engine,
                                ins=[], outs=[])
                            upd = mybir.SyncUpdate(
                                sync_type="semaphore", id=sem,
                                update_mode="sem-wr-imm", update_value=0)
                            on_wait = (list(si.on_wait)
                                       if (k == 0 and si is not None and si.on_wait)
                                       else [])
                            ev.sync_info = mybir.SyncInfo(
                                on_wait=on_wait, on_update=[upd])
                            newlist.append(ev)
                        if si is not None and si.on_update:
                            evf = mybir.InstEventSemaphore(
                                name=f"{ins.name}-clrf", engine=ins.engine,
                                ins=[], outs=[])
                            evf.sync_info = mybir.SyncInfo(
                                on_wait=[], on_update=list(si.on_update))
                            newlist.append(evf)
                    changed = True
                    continue
                si = ins.sync_info
                if (si is not None and si.on_wait is not None
                        and len(si.on_wait) > 1):
                    waits = list(si.on_wait)
                    for i, w in enumerate(waits[1:]):
                        ev = mybir.InstEventSemaphore(
                            name=f"{ins.name}-hw{i}", engine=ins.engine,
                            ins=[], outs=[])
                        ev.sync_info = mybir.SyncInfo(on_wait=[w], on_update=[])
                        newlist.append(ev)
                    si.on_wait = [waits[0]]
                    ins.sync_info = si
                    changed = True
                newlist.append(ins)
            if changed:
                bb.instructions = newlist
    return nc


def build_bass(n_tok_per_batch, n_cores=NCORES):
    N = n_tok_per_batch
    T = B * N
    NT = T // 128             # token tiles (32)
    KB = N // 128             # key tiles per batch (16)

    nc = bass.Bass(trn_type="TRN2", num_devices=n_cores)

    x = nc.dram_tensor("x", [T, C], BF16, kind="ExternalInput")
    w_all = nc.dram_tensor("w_all", [C, QKVW], BF16, kind="ExternalInput")
    b_all = nc.dram_tensor("b_all", [1, QKVW], F32, kind="ExternalInput")
    gbe = nc.dram_tensor("gbe", [128, 4], F32, kind="ExternalInput")
    w_o_loc = nc.dram_tensor("w_o_loc", [CL, C], BF16, kind="ExternalInput")
    out_t = nc.dram_tensor("out_t", [C, T], BF16, kind="ExternalOutput")

    with tile.TileContext(nc) as tc:
        _body(tc, x, w_all, b_all, gbe, w_o_loc, out_t,
              N=N, T=T, NT=NT, KB=KB, n_cores=n_cores)
    return _fixup_module(nc)


def _body(tc, x, w_all, b_all, gbe, w_o_loc, out_t, N, T, NT, KB, n_cores):
    nc = tc.nc

    from contextlib import ExitStack
    octx = ExitStack()
    persist = octx.enter_context(tc.tile_pool(name="persist", bufs=1))

    GB = 4                       # token tiles per phase-1 group
    NG = NT // GB                # 8 groups
    NCH = 2                      # AllReduce chunks (chunk == batch)
    TCH = NT // NCH              # 16 tiles per chunk

    w_all_sb = persist.tile([128, KT, QKVW], BF16)
    for kt in range(KT):
        nc.sync.dma_start(out=w_all_sb[:, kt, :],
                          in_=w_all[kt * 128:(kt + 1) * 128, :])
    b_sb = persist.tile([128, QKVW], F32)
    nc.sync.dma_start(out=b_sb, in_=b_all.ap().to_broadcast([128, QKVW]))
    gbe_sb = persist.tile([128, 4], F32)
    nc.sync.dma_start(out=gbe_sb, in_=gbe[:, :])
    w_o_sb = persist.tile([128, C], BF16)
    nc.sync.dma_start(out=w_o_sb, in_=w_o_loc[:, :])

    qT = persist.tile([128, T], BF16)       # [local col, token]
    kTt = persist.tile([128, T], BF16)
    v_aug = persist.tile([128, NT, 130], BF16)  # [tok%128, tile, 2x(64 v + 1)]
    q_pre = persist.tile([128, NT, 128], BF16)  # [tok%128, tile, local col]
    k_pre = persist.tile([128, NT, 128], BF16)
    # stats cols: 0=sum_q, 1=sum_k, 2=ssq_q, 3=ssq_k
    stats = persist.tile([128, NCH, TCH, 4], F32)
    stats_all = persist.tile([128, NCH, TCH, 4], F32)
    o_un = persist.tile([128, 2 * B * HL, 1024], BF16)  # [dim(65), slot, qtok]
    onorm = persist.tile([128, T], BF16)
    siluo = persist.tile([128, T], BF16)
    scr = persist.tile([128, 128], BF16)

    ones_col = persist.tile([128, NT], F32)
    nc.vector.memset(ones_col, 1.0)
    nc.vector.tensor_copy(out=v_aug[:, :, 64:65], in_=ones_col)
    nc.vector.tensor_copy(out=v_aug[:, :, 129:130], in_=ones_col)

    dram = octx.enter_context(tc.tile_pool(name="dram", bufs=1, space="DRAM"))
    cc_in = [dram.tile([128, TCH * 4], F32, name=f"cc_in{c}")
             for c in range(NCH)]
    cc_out = [dram.tile([128, TCH * 4], F32, name=f"cc_out{c}",
                        addr_space="Shared")
              for c in range(NCH)]

    ph1 = octx.enter_context(tc.tile_pool(name="ph1", bufs=3))
    ph1t = octx.enter_context(tc.tile_pool(name="ph1t", bufs=4))
    ph1s = octx.enter_context(tc.tile_pool(name="ph1s", bufs=4))
    ph2 = octx.enter_context(tc.tile_pool(name="ph2", bufs=1))
    ph3 = octx.enter_context(tc.tile_pool(name="ph3", bufs=8))

    pctx = ExitStack()           # phase-1 PSUM, closed before attention PSUM
    ph1q = pctx.enter_context(tc.tile_pool(name="ph1q", bufs=3, space="PSUM"))

    # ---------------- phase 1: x LN+silu, XBAR transpose, QKV ----------------
    def phase1_group(g):
        xg = ph1.tile([128, GB, C], BF16, name="xg")
        nc.sync.dma_start(
            out=xg,
            in_=x[g * GB * 128:(g + 1) * GB * 128, :].rearrange(
                "(t p) c -> p t c", p=128))

        stats6 = ph1s.tile([128, GB, 2, 6], F32, name="stats6")
        for t in range(GB):
            for h2 in range(2):
                nc.vector.bn_stats(out=stats6[:, t, h2, :],
                                   in_=xg[:, t, h2 * 512:(h2 + 1) * 512])
        mv = ph1s.tile([128, GB, 2], F32, name="mv")
        for t in range(GB):
            nc.vector.bn_aggr(out=mv[:, t, :], in_=stats6[:, t, :, :])

        vpe = ph1s.tile([128, GB, 1], F32, name="vpe")
        nc.vector.tensor_scalar(out=vpe, in0=mv[:, :, 1:2], scalar1=EPS,
                                scalar2=None, op0=ALU.add)
        rstd = _quake_rsqrt(nc, ph1s, vpe, (128, GB, 1), iters=2, suffix="x")
        nmr = ph1s.tile([128, GB, 1], F32, name="nmr")
        nc.vector.tensor_tensor(out=nmr, in0=mv[:, :, 0:1], in1=rstd,
                                op=ALU.mult)
        nc.vector.tensor_scalar(out=nmr, in0=nmr, scalar1=-1.0,
                                scalar2=None, op0=ALU.mult)

        for t in range(GB):
            tt = g * GB + t
            ch = tt // TCH
            ti = tt % TCH
            # silu(LN(x)) in place on ACT
            nc.scalar.activation(out=xg[:, t, :], in_=xg[:, t, :],
                                 func=AF.Silu,
                                 bias=nmr[:, t, :],
                                 scale=rstd[:, t, :])
            # x^T via DMA XBAR: [tok, 128-chunk] -> [ch, tok] per kt
            xsT = ph1t.tile([128, KT, 128], BF16, name="xsT")
            for kt in range(KT):
                nc.sync.dma_start_transpose(
                    out=xsT[:, kt, :], in_=xg[:, t, kt * 128:(kt + 1) * 128])

            pqkv = ph1q.tile([128, QKVW], F32, name="pqkv")
            for kt in range(KT):
                nc.tensor.matmul(
                    pqkv,
                    lhsT=xsT[:, kt, :],
                    rhs=w_all_sb[:, kt, :],
                    start=(kt == 0), stop=(kt == KT - 1))

            # evictions (PSUM fp32 -> SBUF bf16) + bias
            nc.vector.scalar_tensor_tensor(
                out=q_pre[:, tt, :], in0=pqkv[:, 0:128], scalar=1.0,
                in1=b_sb[:, 0:128], op0=ALU.mult, op1=ALU.add)
            nc.vector.scalar_tensor_tensor(
                out=k_pre[:, tt, :], in0=pqkv[:, 128:256], scalar=1.0,
                in1=b_sb[:, 128:256], op0=ALU.mult, op1=ALU.add)
            nc.vector.scalar_tensor_tensor(
                out=v_aug[:, tt, :].rearrange("p (h e) -> p h e", e=65)[:, :, 0:64],
                in0=pqkv[:, 256:384].rearrange("p (h e) -> p h e", e=64),
                scalar=1.0,
                in1=b_sb[:, 256:384].rearrange("p (h e) -> p h e", e=64),
                op0=ALU.mult, op1=ALU.add)
            # q/k sums rode the matmul in the 2 extra weight columns
            # (+ sum-of-local-bias constant from b_sb)
            nc.vector.scalar_tensor_tensor(
                out=stats[:, ch, ti, 0:2], in0=pqkv[:, QKV:QKV + 2],
                scalar=1.0, in1=b_sb[:, QKV:QKV + 2],
                op0=ALU.mult, op1=ALU.add)
            # sums of squares on ACT
            nc.scalar.activation(
                out=scr, in_=q_pre[:, tt, :], func=AF.Square,
                accum_out=stats[:, ch, ti, 2:3])
            nc.scalar.activation(
                out=scr, in_=k_pre[:, tt, :], func=AF.Square,
                accum_out=stats[:, ch, ti, 3:4])

    def emit_allreduce(ch):
        nc.sync.dma_start(out=cc_in[ch],
                          in_=stats[:, ch].rearrange("p a b -> p (a b)"))
        nc.gpsimd.collective_compute(
            "AllReduce", ALU.add,
            replica_groups=[list(range(n_cores))],
            ins=[cc_in[ch].opt()], outs=[cc_out[ch].opt()])
        nc.sync.dma_start(
            out=stats_all[:, ch].rearrange("p a b -> p (a b)"),
            in_=cc_out[ch])

    # phase 2+3 for one chunk: full-inner LN stats -> normalize -> transpose
    def phase23_chunk(ch):
        qk_sn = []
        for which in range(2):  # 0 -> q, 1 -> k
            s_sum = stats_all[:, ch, :, which]
            s_ssq = stats_all[:, ch, :, 2 + which]
            m = ph2.tile([128, TCH], F32, name=f"m_{ch}_{which}")
            nc.vector.tensor_scalar(out=m, in0=s_sum, scalar1=1.0 / INNER,
                                    scalar2=None, op0=ALU.mult)
            msq = ph2.tile([128, TCH], F32, name=f"msq_{ch}_{which}")
            nc.vector.tensor_scalar(out=msq, in0=s_ssq, scalar1=1.0 / INNER,
                                    scalar2=None, op0=ALU.mult)
            tmp = ph2.tile([128, TCH], F32, name=f"tmp_{ch}_{which}")
            nc.vector.tensor_tensor(out=tmp, in0=m, in1=m, op=ALU.mult)
            nc.vector.tensor_tensor(out=tmp, in0=msq, in1=tmp, op=ALU.subtract)
            nc.vector.tensor_scalar(out=tmp, in0=tmp, scalar1=EPS,
                                    scalar2=None, op0=ALU.add)
            rstd = _quake_rsqrt(nc, ph2, tmp, (128, TCH),
                                suffix=f"_{ch}_{which}")
            nmr = ph2.tile([128, TCH], F32, name=f"nmr_{ch}_{which}")
            nc.vector.tensor_tensor(out=nmr, in0=m, in1=rstd, op=ALU.mult)
            nc.vector.tensor_scalar(out=nmr, in0=nmr, scalar1=-1.0,
                                    scalar2=None, op0=ALU.mult)
            qk_sn.append((m, rstd, nmr))

        for ti in range(TCH):
            tt = ch * TCH + ti
            (mq, rq, nq) = qk_sn[0]
            (mk, rk, nk) = qk_sn[1]
            qn = ph3.tile([128, 128], BF16, name="qn")
            nc.vector.tensor_scalar(
                out=qn, in0=q_pre[:, tt, :],
                scalar1=mq[:, ti:ti + 1], scalar2=rq[:, ti:ti + 1],
                op0=ALU.subtract, op1=ALU.mult)
            kn = ph3.tile([128, 128], BF16, name="kn")
            nc.scalar.activation(
                out=kn, in_=k_pre[:, tt, :], func=AF.Identity,
                bias=nk[:, ti:ti + 1], scale=rk[:, ti:ti + 1])
            nc.sync.dma_start_transpose(
                out=qT[:, tt * 128:(tt + 1) * 128], in_=qn)
            nc.sync.dma_start_transpose(
                out=kTt[:, tt * 128:(tt + 1) * 128], in_=kn)
        # gain/beta (and inner**-0.5 on q, folded on host), per-partition
        lo, hi = ch * TCH * 128, (ch + 1) * TCH * 128
        nc.vector.tensor_scalar(
            out=qT[:, lo:hi], in0=qT[:, lo:hi],
            scalar1=gbe_sb[:, 0:1], scalar2=gbe_sb[:, 1:2],
            op0=ALU.mult, op1=ALU.add)
        nc.scalar.activation(
            out=kTt[:, lo:hi], in_=kTt[:, lo:hi], func=AF.Identity,
            bias=gbe_sb[:, 3:4], scale=gbe_sb[:, 2:3])

    # ---------------- phase 4: attention ----------------
    att = octx.enter_context(tc.tile_pool(name="att", bufs=3))
    dramsc = octx.enter_context(tc.tile_pool(name="dramsc", bufs=2,
                                             space="DRAM"))
    dnp = octx.enter_context(tc.tile_pool(name="dnp", bufs=2))
    actx = ExitStack()           # attention PSUM, closed before phase-5 PSUM

    def attention_bh(b, h, attp, attpo):
        for g in range(2):       # q-chunk group: tokens [g*1024, (g+1)*1024)
            slot = b * 4 + h * 2 + g
            pO = attpo.tile([128, 1024], F32, name="pO", tag="pO")
            pend = None          # software pipeline: delay PV by one kb
            for kb in range(KB):
                vt = b * KB + kb
                pS = attp.tile([128, 1024], F32, name="pS", tag="pS")
                for qi in range(2):
                    q0 = b * N + g * 1024 + qi * 512
                    nc.tensor.matmul(
                        pS[:, qi * 512:(qi + 1) * 512],
                        lhsT=kTt[h * 64:(h + 1) * 64,
                                 b * N + kb * 128:b * N + (kb + 1) * 128],
                        rhs=qT[h * 64:(h + 1) * 64, q0:q0 + 512],
                        start=True, stop=True)
                eS = att.tile([128, 1024], BF16, name="eS")
                nc.scalar.activation(out=eS, in_=pS, func=AF.Exp)
                if pend is not None:
                    peS, pvt, pkb = pend
                    for qi in range(2):
                        nc.tensor.matmul(
                            pO[0:65, qi * 512:(qi + 1) * 512],
                            lhsT=v_aug[:, pvt, h * 65:(h + 1) * 65],
                            rhs=peS[:, qi * 512:(qi + 1) * 512],
                            start=(pkb == 0), stop=False)
                pend = (eS, vt, kb)
            peS, pvt, pkb = pend
            for qi in range(2):
                nc.tensor.matmul(
                    pO[0:65, qi * 512:(qi + 1) * 512],
                    lhsT=v_aug[:, pvt, h * 65:(h + 1) * 65],
                    rhs=peS[:, qi * 512:(qi + 1) * 512],
                    start=(pkb == 0), stop=True)
            # evict unnormalized O + raw denominator row
            nc.vector.tensor_copy(out=o_un[0:65, slot, :], in_=pO[0:65, :])

    def denorm_batch(b):
        # batch b's denominators live in o_un[64, b*4:(b+1)*4, :]
        dn_dram = dramsc.tile([1, 4096], BF16, name="dn_dram")
        nc.sync.dma_start(
            out=dn_dram,
            in_=o_un[64:65, b * 4:(b + 1) * 4, :].rearrange(
                "p a t -> p (a t)"))
        dn_g = dnp.tile([128, 32], BF16, name="dn_g")
        nc.sync.dma_start(
            out=dn_g,
            in_=dn_dram[0:1, :].rearrange("o (p c) -> (o p) c", p=128))
        rdn = dnp.tile([128, 32], BF16, name="rdn")
        with nc.allow_low_precision(reason="softmax denom reciprocal, 2e-2 budget"):
            nc.vector.reciprocal(out=rdn, in_=dn_g)
        rdn_dram = dramsc.tile([1, 4096], BF16, name="rdn_dram")
        nc.sync.dma_start(
            out=rdn_dram[0:1, :].rearrange("o (p c) -> (o p) c", p=128),
            in_=rdn)
        dnb = dnp.tile([64, 4096], BF16, name="dnb")
        nc.sync.dma_start(out=dnb, in_=rdn_dram.to_broadcast([64, 4096]))
        for h in range(HL):
            for g in range(2):
                slot = b * 4 + h * 2 + g
                sg = h * 2 + g
                nc.vector.tensor_tensor(
                    out=onorm[h * 64:(h + 1) * 64,
                              b * N + g * 1024:b * N + (g + 1) * 1024],
                    in0=o_un[0:64, slot, :],
                    in1=dnb[:, sg * 1024:(sg + 1) * 1024],
                    op=ALU.mult)
        nc.scalar.activation(out=siluo[:, b * N:(b + 1) * N],
                             in_=onorm[:, b * N:(b + 1) * N], func=AF.Silu)

    # ---------------- emission schedule ----------------
    phase1_group(0)
    phase1_group(1)
    phase1_group(2)
    phase1_group(3)
    emit_allreduce(0)
    phase1_group(4)
    phase1_group(5)
    phase1_group(6)
    phase23_chunk(0)
    phase1_group(7)
    emit_allreduce(1)
    pctx.close()                 # free phase-1 PSUM banks
    attp = actx.enter_context(tc.tile_pool(name="attp", bufs=2, space="PSUM"))
    attpo = actx.enter_context(tc.tile_pool(name="attpo", bufs=2,
                                            space="PSUM"))
    attention_bh(0, 0, attp, attpo)
    phase23_chunk(1)
    attention_bh(0, 1, attp, attpo)
    denorm_batch(0)
    attention_bh(1, 0, attp, attpo)
    attention_bh(1, 1, attp, attpo)
    denorm_batch(1)
    actx.close()                 # free attention PSUM banks

    # ---------------- phase 5: output projection ----------------
    with tc.tile_pool(name="ph5", bufs=4) as ph5, \
         tc.tile_pool(name="ph5p", bufs=4, space="PSUM") as ph5p:
        for tk in range(T // 512):
            for ct in range(KT):
                po = ph5p.tile([128, 512], F32, name="po")
                nc.tensor.matmul(
                    po,
                    lhsT=w_o_sb[:, ct * 128:(ct + 1) * 128],
                    rhs=siluo[:, tk * 512:(tk + 1) * 512],
                    start=True, stop=True)
                ev = ph5.tile([128, 512], BF16, name="ev")
                if (tk * KT + ct) % 2 == 0:
                    nc.vector.tensor_copy(out=ev, in_=po)
                else:
                    nc.scalar.copy(out=ev, in_=po)
                nc.sync.dma_start(
                    out=out_t[ct * 128:(ct + 1) * 128,
                              tk * 512:(tk + 1) * 512],
                    in_=ev)

    octx.close()


def make_in_maps(inputs, n_tok_per_batch, n_cores=NCORES):
    """Slice full inputs into per-core input maps (head sharding)."""
    import ml_dtypes
    bf16 = ml_dtypes.bfloat16

    x = np.ascontiguousarray(np.asarray(inputs["x"], np.float32)
                             .reshape(B * n_tok_per_batch, C)).astype(bf16)
    w_q = np.asarray(inputs["w_q"], np.float32)
    w_k = np.asarray(inputs["w_k"], np.float32)
    w_v = np.asarray(inputs["w_v"], np.float32)
    b_q = np.asarray(inputs["b_q"], np.float32)
    b_k = np.asarray(inputs["b_k"], np.float32)
    b_v = np.asarray(inputs["b_v"], np.float32)
    g_q = np.asarray(inputs["g_q"], np.float32)
    be_q = np.asarray(inputs["be_q"], np.float32)
    g_k = np.asarray(inputs["g_k"], np.float32)
    be_k = np.asarray(inputs["be_k"], np.float32)
    w_o = np.asarray(inputs["w_o"], np.float32)

    scale = float(INNER) ** -0.5
    in_maps = []
    for c in range(n_cores):
        cols = slice(c * CL, (c + 1) * CL)
        wq_l = w_q[:, cols]
        wk_l = w_k[:, cols]
        wv_l = w_v[:, cols]
        w_all = np.ascontiguousarray(np.concatenate(
            [wq_l, wk_l, wv_l,
             wq_l.sum(axis=1, keepdims=True),
             wk_l.sum(axis=1, keepdims=True)], axis=1)).astype(bf16)
        b_all = np.ascontiguousarray(
            np.concatenate([b_q[cols], b_k[cols], b_v[cols],
                            [b_q[cols].sum()], [b_k[cols].sum()]])[None, :]
        ).astype(np.float32)
        gbe = np.ascontiguousarray(
            np.stack([g_q[cols] * scale, be_q[cols] * scale,
                      g_k[cols], be_k[cols]], axis=1))
        w_o_c = np.ascontiguousarray(w_o[cols, :]).astype(bf16)
        in_maps.append({
            "x": x, "w_all": w_all, "b_all": b_all,
            "gbe": gbe, "w_o_loc": w_o_c,
        })
    return in_maps


def combine_outputs(out_ts, inputs, n_tok_per_batch):
    b_o = np.asarray(inputs["b_o"], np.float32)
    acc = np.zeros(out_ts[0].shape, dtype=np.float32)
    for o in out_ts:
        acc += np.asarray(o, dtype=np.float32)
    out = acc.T + b_o[None, :]
    return out.reshape(B, n_tok_per_batch, C).astype(np.float32)


_NC_CACHE = {}


def kernel(**inputs):
    from concourse.bass_utils import run_bass_kernel_spmd

    n_tok = np.asarray(inputs["x"]).shape[1]
    if n_tok not in _NC_CACHE:
        _NC_CACHE[n_tok] = build_bass(n_tok)
    nc = _NC_CACHE[n_tok]
    in_maps = make_in_maps(inputs, n_tok)
    res = run_bass_kernel_spmd(nc, in_maps, core_ids=list(range(NCORES)))
    out_ts = [r["out_t"] for r in res.results]
    return combine_outputs(out_ts, inputs, n_tok)


# revision 8
# speedup vs baseline: 1.3748x; 1.3748x over previous
"""Trainium2 Bass kernel for nn_Attention_71846212928150.

Self-attention block (pre-LN + silu, QKV projections, per-head attention with
q/k LayerNorms, output projection), sharded over 8 NeuronCores by heads:
core c owns heads {2c, 2c+1} = inner columns [128c, 128c+128).

v2 design (vs. the fp32r v1): all PE operands are bf16 (PSUM accumulation
stays fp32), all transposes run on the DMA XBAR (dma_start_transpose) instead
of the PE, q/k LN sums ride the QKV matmul as two host-precomputed row-sum
weight columns, the stats AllReduce is split into two chunks overlapped with
compute, the attention loop is software-pipelined (S(kb+1) issued before
PV(kb)) with double-buffered PSUM so the PE never idles, and the softmax
denominators are batched into a [128, 32] reciprocal instead of a 1-partition
15us DVE reciprocal per (batch, head).
"""

import numpy as np

import concourse.bass as bass
import concourse.mybir as mybir
import concourse.tile as tile

F32 = mybir.dt.float32
BF16 = mybir.dt.bfloat16
I32 = mybir.dt.int32
AF = mybir.ActivationFunctionType
ALU = mybir.AluOpType
AX = mybir.AxisListType

B = 2
C = 1024
H = 16
DH = 64
INNER = H * DH
NCORES = 8
HL = H // NCORES          # 2 heads per core
CL = HL * DH              # 128 local inner columns
QKV = 3 * CL              # 384
QKVW = QKV + 2            # + sum_q / sum_k stat columns
KT = C // 128             # 8 contraction tiles over C
EPS = 1e-5
MAGIC = 0x5F3759DF


def _quake_rsqrt(nc, pool, vpe, shape, iters=3, suffix=""):
    """rstd = 1/sqrt(vpe) entirely on DVE (fp32 bitcast + Newton steps)."""
    y = pool.tile(list(shape), F32, name=f"qk_y{suffix}")
    t2 = pool.tile(list(shape), F32, name=f"qk_t2{suffix}")
    nc.vector.tensor_scalar(
        out=y.bitcast(I32), in0=vpe.bitcast(I32), scalar1=1, scalar2=None,
        op0=ALU.logical_shift_right)
    nc.vector.tensor_scalar(
        out=y.bitcast(I32), in0=y.bitcast(I32), scalar1=-1, scalar2=MAGIC,
        op0=ALU.mult, op1=ALU.add)
    for _ in range(iters):
        nc.vector.tensor_tensor(out=t2, in0=y, in1=y, op=ALU.mult)
        nc.vector.tensor_tensor(out=t2, in0=t2, in1=vpe, op=ALU.mult)
        nc.vector.tensor_scalar(out=t2, in0=t2, scalar1=-0.5, scalar2=1.5,
                                op0=ALU.mult, op1=ALU.add)
        nc.vector.tensor_tensor(out=y, in0=y, in1=t2, op=ALU.mult)
    return y


def _fixup_module(nc):
    """Adapt Tile-emitted BIR to this container's walrus build.

    1. The tail `EVENT_SEMAPHORE_RANGE_CLEAR` InstISA (opcode 176) is not
       understood by this walrus' birverifier. Replace it with one
       EventSemaphore sem-write-0 per semaphore in the cleared range.
    2. Drain instructions carrying more than one semaphore wait fail codegen;
       hoist the extra waits into standalone EventSemaphore waits.
    """
    for f in nc.m.functions:
        for bb in f.blocks:
            newlist = []
            changed = False
            for ins in bb.instructions:
                tn = type(ins).__name__
                if tn == "InstISA" and getattr(ins, "isa_opcode", None) == 176:
                    ad = ins.ant_dict or {}
                    first = ad.get("range_first")
                    last = ad.get("range_last")
                    if first is not None and last is not None:
                        si = ins.sync_info
                        sems = list(range(first, last + 1))
                        for k, sem in enumerate(sems):
                            ev = mybir.InstEventSemaphore(
                                name=f"{ins.name}-clr{k}", engine=ins.engine,
                                ins=[], outs=[])
                            upd = mybir.SyncUpdate(
                                sync_type="semaphore", id=sem,
                                update_mode="sem-wr-imm", update_value=0)
                            on_wait = (list(si.on_wait)
                                       if (k == 0 and si is not None and si.on_wait)
                                       else [])
                            ev.sync_info = mybir.SyncInfo(
                                on_wait=on_wait, on_update=[upd])
                            newlist.append(ev)
                        if si is not None and si.on_update:
                            evf = mybir.InstEventSemaphore(
                                name=f"{ins.name}-clrf", engine=ins.engine,
                                ins=[], outs=[])
                            evf.sync_info = mybir.SyncInfo(
                                on_wait=[], on_update=list(si.on_update))
                            newlist.append(evf)
                    changed = True
                    continue
                si = ins.sync_info
                if (si is not None and si.on_wait is not None
                        and len(si.on_wait) > 1):
                    waits = list(si.on_wait)
                    for i, w in enumerate(waits[1:]):
                        ev = mybir.InstEventSemaphore(
                            name=f"{ins.name}-hw{i}", engine=ins.engine,
                            ins=[], outs=[])
                        ev.sync_info = mybir.SyncInfo(on_wait=[w], on_update=[])
                        newlist.append(ev)
                    si.on_wait = [waits[0]]
                    ins.sync_info = si
                    changed = True
                newlist.append(ins)
            if changed:
                bb.instructions = newlist
    return nc


def build_bass(n_tok_per_batch, n_cores=NCORES):
    N = n_tok_per_batch
    T = B * N
    NT = T // 128             # token tiles (32)
    KB = N // 128             # key tiles per batch (16)

    nc = bass.Bass(trn_type="TRN2", num_devices=n_cores)

    x = nc.dram_tensor("x", [T, C], BF16, kind="ExternalInput")
    w_all = nc.dram_tensor("w_all", [C, QKVW], BF16, kind="ExternalInput")
    b_all = nc.dram_tensor("b_all", [1, QKVW], F32, kind="ExternalInput")
    gbe = nc.dram_tensor("gbe", [128, 4], F32, kind="ExternalInput")
    w_o_loc = nc.dram_tensor("w_o_loc", [CL, C], BF16, kind="ExternalInput")
    out_t = nc.dram_tensor("out_t", [C, T], BF16, kind="ExternalOutput")

    with tile.TileContext(nc) as tc:
        _body(tc, x, w_all, b_all, gbe, w_o_loc, out_t,
              N=N, T=T, NT=NT, KB=KB, n_cores=n_cores)
    return _fixup_module(nc)


def _body(tc, x, w_all, b_all, gbe, w_o_loc, out_t, N, T, NT, KB, n_cores):
    nc = tc.nc

    from contextlib import ExitStack
    octx = ExitStack()
    persist = octx.enter_context(tc.tile_pool(name="persist", bufs=1))

    GB = 4                       # token tiles per phase-1 group
    NG = NT // GB                # 8 groups
    NCH = 2                      # AllReduce chunks (chunk == batch)
    TCH = NT // NCH              # 16 tiles per chunk

    w_all_sb = persist.tile([128, KT, QKVW], BF16)
    for kt in range(KT):
        nc.sync.dma_start(out=w_all_sb[:, kt, :],
                          in_=w_all[kt * 128:(kt + 1) * 128, :])
    b_sb = persist.tile([128, QKVW], F32)
    nc.sync.dma_start(out=b_sb, in_=b_all.ap().to_broadcast([128, QKVW]))
    gbe_sb = persist.tile([128, 4], F32)
    nc.sync.dma_start(out=gbe_sb, in_=gbe[:, :])
    w_o_sb = persist.tile([128, C], BF16)
    nc.sync.dma_start(out=w_o_sb, in_=w_o_loc[:, :])

    qT = persist.tile([128, T], BF16)       # [local col, token]
    kTt = persist.tile([128, T], BF16)
    v_aug = persist.tile([128, NT, 130], BF16)  # [tok%128, tile, 2x(64 v + 1)]
    q_pre = persist.tile([128, NT, 128], BF16)  # [tok%128, tile, local col]
    k_pre = persist.tile([128, NT, 128], BF16)
    # stats cols: 0=sum_q, 1=sum_k, 2=ssq_q, 3=ssq_k
    stats = persist.tile([128, NCH, TCH, 4], F32)
    stats_all = persist.tile([128, NCH, TCH, 4], F32)
    o_un = persist.tile([128, 2 * B * HL, 1024], BF16)  # [dim(65), slot, qtok]
    onorm = persist.tile([128, T], BF16)
    siluo = persist.tile([128, T], BF16)
    scr = persist.tile([128, 128], BF16)

    ones_col = persist.tile([128, NT], F32)
    nc.vector.memset(ones_col, 1.0)
    nc.vector.tensor_copy(out=v_aug[:, :, 64:65], in_=ones_col)
    nc.vector.tensor_copy(out=v_aug[:, :, 129:130], in_=ones_col)

    dram = octx.enter_context(tc.tile_pool(name="dram", bufs=1, space="DRAM"))
    cc_in = [dram.tile([128, TCH * 4], F32, name=f"cc_in{c}")
             for c in range(NCH)]
    cc_out = [dram.tile([128, TCH * 4], F32, name=f"cc_out{c}",
                        addr_space="Shared")
              for c in range(NCH)]
    # start barrier: absorbs the cross-core launch skew before any real work,
    # so the stats AllReduces later see aligned peers
    bar_in = dram.tile([1, 4], F32, name="bar_in")
    bar_out = dram.tile([1, 4], F32, name="bar_out", addr_space="Shared")
    nc.gpsimd.collective_compute(
        "AllReduce", ALU.add,
        replica_groups=[list(range(n_cores))],
        ins=[bar_in.opt()], outs=[bar_out.opt()])

    ph1 = octx.enter_context(tc.tile_pool(name="ph1", bufs=3))
    ph1t = octx.enter_context(tc.tile_pool(name="ph1t", bufs=4))
    ph1s = octx.enter_context(tc.tile_pool(name="ph1s", bufs=4))
    ph2 = octx.enter_context(tc.tile_pool(name="ph2", bufs=1))
    ph3 = octx.enter_context(tc.tile_pool(name="ph3", bufs=8))

    pctx = ExitStack()           # phase-1 PSUM, closed before attention PSUM
    ph1q = pctx.enter_context(tc.tile_pool(name="ph1q", bufs=3, space="PSUM"))

    # ---------------- phase 1: x LN+silu, XBAR transpose, QKV ----------------
    def phase1_group(g):
        xg = ph1.tile([128, GB, C], BF16, name="xg")
        nc.sync.dma_start(
            out=xg,
            in_=x[g * GB * 128:(g + 1) * GB * 128, :].rearrange(
                "(t p) c -> p t c", p=128))

        stats6 = ph1s.tile([128, GB, 2, 6], F32, name="stats6")
        for t in range(GB):
            for h2 in range(2):
                nc.vector.bn_stats(out=stats6[:, t, h2, :],
                                   in_=xg[:, t, h2 * 512:(h2 + 1) * 512])
        mv = ph1s.tile([128, GB, 2], F32, name="mv")
        for t in range(GB):
            nc.vector.bn_aggr(out=mv[:, t, :], in_=stats6[:, t, :, :])

        vpe = ph1s.tile([128, GB, 1], F32, name="vpe")
        nc.vector.tensor_scalar(out=vpe, in0=mv[:, :, 1:2], scalar1=EPS,
                                scalar2=None, op0=ALU.add)
        rstd = _quake_rsqrt(nc, ph1s, vpe, (128, GB, 1), iters=2, suffix="x")
        nmr = ph1s.tile([128, GB, 1], F32, name="nmr")
        nc.vector.tensor_tensor(out=nmr, in0=mv[:, :, 0:1], in1=rstd,
                                op=ALU.mult)
        nc.vector.tensor_scalar(out=nmr, in0=nmr, scalar1=-1.0,
                                scalar2=None, op0=ALU.mult)

        for t in range(GB):
            tt = g * GB + t
            ch = tt // TCH
            ti = tt % TCH
            # silu(LN(x)) in place on ACT
            nc.scalar.activation(out=xg[:, t, :], in_=xg[:, t, :],
                                 func=AF.Silu,
                                 bias=nmr[:, t, :],
                                 scale=rstd[:, t, :])
            # x^T via one DMA XBAR op: [tok, 1024] -> [ch%128, ch//128, tok]
            xsT = ph1t.tile([128, KT, 128], BF16, name="xsT")
            nc.sync.dma_start_transpose(out=xsT, in_=xg[:, t, :])

            pqkv = ph1q.tile([128, QKVW], F32, name="pqkv")
            for kt in range(KT):
                nc.tensor.matmul(
                    pqkv,
                    lhsT=xsT[:, kt, :],
                    rhs=w_all_sb[:, kt, :],
                    start=(kt == 0), stop=(kt == KT - 1))

            # evictions (PSUM fp32 -> SBUF bf16) + bias
            nc.vector.scalar_tensor_tensor(
                out=q_pre[:, tt, :], in0=pqkv[:, 0:128], scalar=1.0,
                in1=b_sb[:, 0:128], op0=ALU.mult, op1=ALU.add)
            nc.vector.scalar_tensor_tensor(
                out=k_pre[:, tt, :], in0=pqkv[:, 128:256], scalar=1.0,
                in1=b_sb[:, 128:256], op0=ALU.mult, op1=ALU.add)
            nc.vector.scalar_tensor_tensor(
                out=v_aug[:, tt, :].rearrange("p (h e) -> p h e", e=65)[:, :, 0:64],
                in0=pqkv[:, 256:384].rearrange("p (h e) -> p h e", e=64),
                scalar=1.0,
                in1=b_sb[:, 256:384].rearrange("p (h e) -> p h e", e=64),
                op0=ALU.mult, op1=ALU.add)
            # q/k sums rode the matmul in the 2 extra weight columns
            # (+ sum-of-local-bias constant from b_sb)
            nc.vector.scalar_tensor_tensor(
                out=stats[:, ch, ti, 0:2], in0=pqkv[:, QKV:QKV + 2],
                scalar=1.0, in1=b_sb[:, QKV:QKV + 2],
                op0=ALU.mult, op1=ALU.add)
            # sums of squares on ACT
            nc.scalar.activation(
                out=scr, in_=q_pre[:, tt, :], func=AF.Square,
                accum_out=stats[:, ch, ti, 2:3])
            nc.scalar.activation(
                out=scr, in_=k_pre[:, tt, :], func=AF.Square,
                accum_out=stats[:, ch, ti, 3:4])

    def emit_allreduce(ch):
        nc.sync.dma_start(out=cc_in[ch],
                          in_=stats[:, ch].rearrange("p a b -> p (a b)"))
        nc.gpsimd.collective_compute(
            "AllReduce", ALU.add,
            replica_groups=[list(range(n_cores))],
            ins=[cc_in[ch].opt()], outs=[cc_out[ch].opt()])
        nc.sync.dma_start(
            out=stats_all[:, ch].rearrange("p a b -> p (a b)"),
            in_=cc_out[ch])

    # phase 2+3 for one chunk: full-inner LN stats -> normalize -> transpose
    def phase23_chunk(ch):
        qk_sn = []
        for which in range(2):  # 0 -> q, 1 -> k
            s_sum = stats_all[:, ch, :, which]
            s_ssq = stats_all[:, ch, :, 2 + which]
            m = ph2.tile([128, TCH], F32, name=f"m_{ch}_{which}")
            nc.vector.tensor_scalar(out=m, in0=s_sum, scalar1=1.0 / INNER,
                                    scalar2=None, op0=ALU.mult)
            msq = ph2.tile([128, TCH], F32, name=f"msq_{ch}_{which}")
            nc.vector.tensor_scalar(out=msq, in0=s_ssq, scalar1=1.0 / INNER,
                                    scalar2=None, op0=ALU.mult)
            tmp = ph2.tile([128, TCH], F32, name=f"tmp_{ch}_{which}")
            nc.vector.tensor_tensor(out=tmp, in0=m, in1=m, op=ALU.mult)
            nc.vector.tensor_tensor(out=tmp, in0=msq, in1=tmp, op=ALU.subtract)
            nc.vector.tensor_scalar(out=tmp, in0=tmp, scalar1=EPS,
                                    scalar2=None, op0=ALU.add)
            rstd = _quake_rsqrt(nc, ph2, tmp, (128, TCH),
                                suffix=f"_{ch}_{which}")
            nmr = ph2.tile([128, TCH], F32, name=f"nmr_{ch}_{which}")
            nc.vector.tensor_tensor(out=nmr, in0=m, in1=rstd, op=ALU.mult)
            nc.vector.tensor_scalar(out=nmr, in0=nmr, scalar1=-1.0,
                                    scalar2=None, op0=ALU.mult)
            qk_sn.append((m, rstd, nmr))

        for ti in range(TCH):
            tt = ch * TCH + ti
            (mq, rq, nq) = qk_sn[0]
            (mk, rk, nk) = qk_sn[1]
            qn = ph3.tile([128, 128], BF16, name="qn")
            nc.vector.tensor_scalar(
                out=qn, in0=q_pre[:, tt, :],
                scalar1=mq[:, ti:ti + 1], scalar2=rq[:, ti:ti + 1],
                op0=ALU.subtract, op1=ALU.mult)
            kn = ph3.tile([128, 128], BF16, name="kn")
            nc.scalar.activation(
                out=kn, in_=k_pre[:, tt, :], func=AF.Identity,
                bias=nk[:, ti:ti + 1], scale=rk[:, ti:ti + 1])
            nc.sync.dma_start_transpose(
                out=qT[:, tt * 128:(tt + 1) * 128], in_=qn)
            nc.sync.dma_start_transpose(
                out=kTt[:, tt * 128:(tt + 1) * 128], in_=kn)
        # gain/beta (and inner**-0.5 on q, folded on host), per-partition
        lo, hi = ch * TCH * 128, (ch + 1) * TCH * 128
        nc.vector.tensor_scalar(
            out=qT[:, lo:hi], in0=qT[:, lo:hi],
            scalar1=gbe_sb[:, 0:1], scalar2=gbe_sb[:, 1:2],
            op0=ALU.mult, op1=ALU.add)
        nc.scalar.activation(
            out=kTt[:, lo:hi], in_=kTt[:, lo:hi], func=AF.Identity,
            bias=gbe_sb[:, 3:4], scale=gbe_sb[:, 2:3])

    # ---------------- phase 4: attention ----------------
    att = octx.enter_context(tc.tile_pool(name="att", bufs=3))
    dramsc = octx.enter_context(tc.tile_pool(name="dramsc", bufs=2,
                                             space="DRAM"))
    dnp = octx.enter_context(tc.tile_pool(name="dnp", bufs=2))
    actx = ExitStack()           # attention PSUM, closed before phase-5 PSUM

    def attention_bh(b, h, attp, attpo):
        for g in range(2):       # q-chunk group: tokens [g*1024, (g+1)*1024)
            slot = b * 4 + h * 2 + g
            pO = attpo.tile([128, 1024], F32, name="pO", tag="pO")
            pend = None          # software pipeline: delay PV by one kb
            for kb in range(KB):
                vt = b * KB + kb
                pS = attp.tile([128, 1024], F32, name="pS", tag="pS")
                for qi in range(2):
                    q0 = b * N + g * 1024 + qi * 512
                    nc.tensor.matmul(
                        pS[:, qi * 512:(qi + 1) * 512],
                        lhsT=kTt[h * 64:(h + 1) * 64,
                                 b * N + kb * 128:b * N + (kb + 1) * 128],
                        rhs=qT[h * 64:(h + 1) * 64, q0:q0 + 512],
                        start=True, stop=True)
                eS = att.tile([128, 1024], BF16, name="eS")
                nc.scalar.activation(out=eS, in_=pS, func=AF.Exp)
                if pend is not None:
                    peS, pvt, pkb = pend
                    for qi in range(2):
                        nc.tensor.matmul(
                            pO[0:65, qi * 512:(qi + 1) * 512],
                            lhsT=v_aug[:, pvt, h * 65:(h + 1) * 65],
                            rhs=peS[:, qi * 512:(qi + 1) * 512],
                            start=(pkb == 0), stop=False)
                pend = (eS, vt, kb)
            peS, pvt, pkb = pend
            for qi in range(2):
                nc.tensor.matmul(
                    pO[0:65, qi * 512:(qi + 1) * 512],
                    lhsT=v_aug[:, pvt, h * 65:(h + 1) * 65],
                    rhs=peS[:, qi * 512:(qi + 1) * 512],
                    start=(pkb == 0), stop=True)
            # evict unnormalized O + raw denominator row
            nc.vector.tensor_copy(out=o_un[0:65, slot, :], in_=pO[0:65, :])

    def denorm_batch(b):
        # batch b's denominators live in o_un[64, b*4:(b+1)*4, :]
        dn_dram = dramsc.tile([1, 4096], BF16, name="dn_dram")
        nc.sync.dma_start(
            out=dn_dram,
            in_=o_un[64:65, b * 4:(b + 1) * 4, :].rearrange(
                "p a t -> p (a t)"))
        dn_g = dnp.tile([128, 32], BF16, name="dn_g")
        nc.sync.dma_start(
            out=dn_g,
            in_=dn_dram[0:1, :].rearrange("o (p c) -> (o p) c", p=128))
        rdn = dnp.tile([128, 32], BF16, name="rdn")
        with nc.allow_low_precision(reason="softmax denom reciprocal, 2e-2 budget"):
            nc.vector.reciprocal(out=rdn, in_=dn_g)
        rdn_dram = dramsc.tile([1, 4096], BF16, name="rdn_dram")
        nc.sync.dma_start(
            out=rdn_dram[0:1, :].rearrange("o (p c) -> (o p) c", p=128),
            in_=rdn)
        dnb = dnp.tile([64, 4096], BF16, name="dnb")
        nc.sync.dma_start(out=dnb, in_=rdn_dram.to_broadcast([64, 4096]))
        for h in range(HL):
            for g in range(2):
                slot = b * 4 + h * 2 + g
                sg = h * 2 + g
                nc.vector.tensor_tensor(
                    out=onorm[h * 64:(h + 1) * 64,
                              b * N + g * 1024:b * N + (g + 1) * 1024],
                    in0=o_un[0:64, slot, :],
                    in1=dnb[:, sg * 1024:(sg + 1) * 1024],
                    op=ALU.mult)

    def silu_batch(b):
        nc.scalar.activation(out=siluo[:, b * N:(b + 1) * N],
                             in_=onorm[:, b * N:(b + 1) * N], func=AF.Silu)

    # ---------------- emission schedule ----------------
    phase1_group(0)
    phase1_group(1)
    phase1_group(2)
    phase1_group(3)
    emit_allreduce(0)
    phase1_group(4)
    phase1_group(5)
    phase1_group(6)
    phase1_group(7)
    emit_allreduce(1)
    phase23_chunk(0)
    pctx.close()                 # free phase-1 PSUM banks
    attp = actx.enter_context(tc.tile_pool(name="attp", bufs=2, space="PSUM"))
    attpo = actx.enter_context(tc.tile_pool(name="attpo", bufs=2,
                                            space="PSUM"))
    attention_bh(0, 0, attp, attpo)
    phase23_chunk(1)
    attention_bh(0, 1, attp, attpo)
    denorm_batch(0)
    attention_bh(1, 0, attp, attpo)
    silu_batch(0)
    attention_bh(1, 1, attp, attpo)
    denorm_batch(1)
    silu_batch(1)
    actx.close()                 # free attention PSUM banks

    # ---------------- phase 5: output projection ----------------
    with tc.tile_pool(name="ph5", bufs=4) as ph5, \
         tc.tile_pool(name="ph5p", bufs=4, space="PSUM") as ph5p:
        for tk in range(T // 512):
            for ct in range(KT):
                po = ph5p.tile([128, 512], F32, name="po")
                nc.tensor.matmul(
                    po,
                    lhsT=w_o_sb[:, ct * 128:(ct + 1) * 128],
                    rhs=siluo[:, tk * 512:(tk + 1) * 512],
                    start=True, stop=True)
                ev = ph5.tile([128, 512], BF16, name="ev")
                if (tk * KT + ct) % 2 == 0:
                    nc.vector.tensor_copy(out=ev, in_=po)
                else:
                    nc.scalar.copy(out=ev, in_=po)
                nc.sync.dma_start(
                    out=out_t[ct * 128:(ct + 1) * 128,
                              tk * 512:(tk + 1) * 512],
                    in_=ev)

    octx.close()


def make_in_maps(inputs, n_tok_per_batch, n_cores=NCORES):
    """Slice full inputs into per-core input maps (head sharding)."""
    import ml_dtypes
    bf16 = ml_dtypes.bfloat16

    x = np.ascontiguousarray(np.asarray(inputs["x"], np.float32)
                             .reshape(B * n_tok_per_batch, C)).astype(bf16)
    w_q = np.asarray(inputs["w_q"], np.float32)
    w_k = np.asarray(inputs["w_k"], np.float32)
    w_v = np.asarray(inputs["w_v"], np.float32)
    b_q = np.asarray(inputs["b_q"], np.float32)
    b_k = np.asarray(inputs["b_k"], np.float32)
    b_v = np.asarray(inputs["b_v"], np.float32)
    g_q = np.asarray(inputs["g_q"], np.float32)
    be_q = np.asarray(inputs["be_q"], np.float32)
    g_k = np.asarray(inputs["g_k"], np.float32)
    be_k = np.asarray(inputs["be_k"], np.float32)
    w_o = np.asarray(inputs["w_o"], np.float32)

    scale = float(INNER) ** -0.5
    in_maps = []
    for c in range(n_cores):
        cols = slice(c * CL, (c + 1) * CL)
        wq_l = w_q[:, cols]
        wk_l = w_k[:, cols]
        wv_l = w_v[:, cols]
        w_all = np.ascontiguousarray(np.concatenate(
            [wq_l, wk_l, wv_l,
             wq_l.sum(axis=1, keepdims=True),
             wk_l.sum(axis=1, keepdims=True)], axis=1)).astype(bf16)
        b_all = np.ascontiguousarray(
            np.concatenate([b_q[cols], b_k[cols], b_v[cols],
                            [b_q[cols].sum()], [b_k[cols].sum()]])[None, :]
        ).astype(np.float32)
        gbe = np.ascontiguousarray(
            np.stack([g_q[cols] * scale, be_q[cols] * scale,
                      g_k[cols], be_k[cols]], axis=1))
        w_o_c = np.ascontiguousarray(w_o[cols, :]).astype(bf16)
        in_maps.append({
            "x": x, "w_all": w_all, "b_all": b_all,
            "gbe": gbe, "w_o_loc": w_o_c,
        })
    return in_maps


def combine_outputs(out_ts, inputs, n_tok_per_batch):
    b_o = np.asarray(inputs["b_o"], np.float32)
    acc = np.zeros(out_ts[0].shape, dtype=np.float32)
    for o in out_ts:
        acc += np.asarray(o, dtype=np.float32)
    out = acc.T + b_o[None, :]
    return out.reshape(B, n_tok_per_batch, C).astype(np.float32)


_NC_CACHE = {}


def kernel(**inputs):
    from concourse.bass_utils import run_bass_kernel_spmd

    n_tok = np.asarray(inputs["x"]).shape[1]
    if n_tok not in _NC_CACHE:
        _NC_CACHE[n_tok] = build_bass(n_tok)
    nc = _NC_CACHE[n_tok]
    in_maps = make_in_maps(inputs, n_tok)
    res = run_bass_kernel_spmd(nc, in_maps, core_ids=list(range(NCORES)))
    out_ts = [r["out_t"] for r in res.results]
    return combine_outputs(out_ts, inputs, n_tok)


# revision 18
# speedup vs baseline: 1.6096x; 1.1707x over previous
"""Trainium2 Bass kernel for nn_Attention_71846212928150.

Self-attention block (pre-LN + silu, QKV projections, per-head attention with
q/k LayerNorms, output projection), sharded over 8 NeuronCores by heads:
core c owns heads {2c, 2c+1} = inner columns [128c, 128c+128).

v2 design (vs. the fp32r v1): all PE operands are bf16 (PSUM accumulation
stays fp32), all transposes run on the DMA XBAR (dma_start_transpose) instead
of the PE, q/k LN sums ride the QKV matmul as two host-precomputed row-sum
weight columns, the stats AllReduce is split into two chunks overlapped with
compute, the attention loop is software-pipelined (S(kb+1) issued before
PV(kb)) with double-buffered PSUM so the PE never idles, and the softmax
denominators are batched into a [128, 32] reciprocal instead of a 1-partition
15us DVE reciprocal per (batch, head).
"""

import numpy as np

import concourse.bass as bass
import concourse.mybir as mybir
import concourse.tile as tile

F32 = mybir.dt.float32
BF16 = mybir.dt.bfloat16
FP8 = mybir.dt.float8e4
I32 = mybir.dt.int32
AF = mybir.ActivationFunctionType
ALU = mybir.AluOpType
AX = mybir.AxisListType

B = 2
C = 1024
H = 16
DH = 64
INNER = H * DH
NCORES = 8
HL = H // NCORES          # 2 heads per core
CL = HL * DH              # 128 local inner columns
QKV = 3 * CL              # 384
QKVW = QKV + 2            # + sum_q / sum_k stat columns
KT = C // 128             # 8 contraction tiles over C
EPS = 1e-5
MAGIC = 0x5F3759DF


def _quake_rsqrt(nc, pool, vpe, shape, iters=3, suffix=""):
    """rstd = 1/sqrt(vpe) entirely on DVE (fp32 bitcast + Newton steps)."""
    y = pool.tile(list(shape), F32, name=f"qk_y{suffix}")
    t2 = pool.tile(list(shape), F32, name=f"qk_t2{suffix}")
    nc.vector.tensor_scalar(
        out=y.bitcast(I32), in0=vpe.bitcast(I32), scalar1=1, scalar2=None,
        op0=ALU.logical_shift_right)
    nc.vector.tensor_scalar(
        out=y.bitcast(I32), in0=y.bitcast(I32), scalar1=-1, scalar2=MAGIC,
        op0=ALU.mult, op1=ALU.add)
    for _ in range(iters):
        nc.vector.tensor_tensor(out=t2, in0=y, in1=y, op=ALU.mult)
        nc.vector.tensor_tensor(out=t2, in0=t2, in1=vpe, op=ALU.mult)
        nc.vector.tensor_scalar(out=t2, in0=t2, scalar1=-0.5, scalar2=1.5,
                                op0=ALU.mult, op1=ALU.add)
        nc.vector.tensor_tensor(out=y, in0=y, in1=t2, op=ALU.mult)
    return y


def _fixup_module(nc):
    """Adapt Tile-emitted BIR to this container's walrus build.

    1. The tail `EVENT_SEMAPHORE_RANGE_CLEAR` InstISA (opcode 176) is not
       understood by this walrus' birverifier. Replace it with one
       EventSemaphore sem-write-0 per semaphore in the cleared range.
    2. Drain instructions carrying more than one semaphore wait fail codegen;
       hoist the extra waits into standalone EventSemaphore waits.
    """
    for f in nc.m.functions:
        for bb in f.blocks:
            newlist = []
            changed = False
            for ins in bb.instructions:
                tn = type(ins).__name__
                if tn == "InstISA" and getattr(ins, "isa_opcode", None) == 176:
                    ad = ins.ant_dict or {}
                    first = ad.get("range_first")
                    last = ad.get("range_last")
                    if first is not None and last is not None:
                        si = ins.sync_info
                        sems = list(range(first, last + 1))
                        for k, sem in enumerate(sems):
                            ev = mybir.InstEventSemaphore(
                                name=f"{ins.name}-clr{k}", engine=ins.engine,
                                ins=[], outs=[])
                            upd = mybir.SyncUpdate(
                                sync_type="semaphore", id=sem,
                                update_mode="sem-wr-imm", update_value=0)
                            on_wait = (list(si.on_wait)
                                       if (k == 0 and si is not None and si.on_wait)
                                       else [])
                            ev.sync_info = mybir.SyncInfo(
                                on_wait=on_wait, on_update=[upd])
                            newlist.append(ev)
                        if si is not None and si.on_update:
                            evf = mybir.InstEventSemaphore(
                                name=f"{ins.name}-clrf", engine=ins.engine,
                                ins=[], outs=[])
                            evf.sync_info = mybir.SyncInfo(
                                on_wait=[], on_update=list(si.on_update))
                            newlist.append(evf)
                    changed = True
                    continue
                si = ins.sync_info
                if (si is not None and si.on_wait is not None
                        and len(si.on_wait) > 1):
                    waits = list(si.on_wait)
                    for i, w in enumerate(waits[1:]):
                        ev = mybir.InstEventSemaphore(
                            name=f"{ins.name}-hw{i}", engine=ins.engine,
                            ins=[], outs=[])
                        ev.sync_info = mybir.SyncInfo(on_wait=[w], on_update=[])
                        newlist.append(ev)
                    si.on_wait = [waits[0]]
                    ins.sync_info = si
                    changed = True
                newlist.append(ins)
            if changed:
                bb.instructions = newlist
    return nc


def build_bass(n_tok_per_batch, n_cores=NCORES):
    N = n_tok_per_batch
    T = B * N
    NT = T // 128             # token tiles (32)
    KB = N // 128             # key tiles per batch (16)

    nc = bass.Bass(trn_type="TRN2", num_devices=n_cores)

    x = nc.dram_tensor("x", [T, C], BF16, kind="ExternalInput")
    w_all = nc.dram_tensor("w_all", [C, QKVW], BF16, kind="ExternalInput")
    b_all = nc.dram_tensor("b_all", [1, QKVW], F32, kind="ExternalInput")
    gbe = nc.dram_tensor("gbe", [128, 4], F32, kind="ExternalInput")
    w_o_loc = nc.dram_tensor("w_o_loc", [CL, C], BF16, kind="ExternalInput")
    out_t = nc.dram_tensor("out_t", [C, T], BF16, kind="ExternalOutput")

    with tile.TileContext(nc) as tc:
        _body(tc, x, w_all, b_all, gbe, w_o_loc, out_t,
              N=N, T=T, NT=NT, KB=KB, n_cores=n_cores)
    return _fixup_module(nc)


def _body(tc, x, w_all, b_all, gbe, w_o_loc, out_t, N, T, NT, KB, n_cores):
    nc = tc.nc

    from contextlib import ExitStack
    octx = ExitStack()
    persist = octx.enter_context(tc.tile_pool(name="persist", bufs=1))

    GB = 4                       # token tiles per phase-1 group
    NG = NT // GB                # 8 groups
    NCH = 2                      # AllReduce chunks (chunk == batch)
    TCH = NT // NCH              # 16 tiles per chunk

    w_all_sb = persist.tile([128, KT, QKVW], BF16)
    for kt in range(KT):
        nc.sync.dma_start(out=w_all_sb[:, kt, :],
                          in_=w_all[kt * 128:(kt + 1) * 128, :])
    b_sb = persist.tile([128, QKVW], F32)
    nc.sync.dma_start(out=b_sb, in_=b_all.ap().to_broadcast([128, QKVW]))
    gbe_sb = persist.tile([128, 4], F32)
    nc.sync.dma_start(out=gbe_sb, in_=gbe[:, :])
    w_o_sb = persist.tile([128, C], BF16)
    nc.sync.dma_start(out=w_o_sb, in_=w_o_loc[:, :])

    qT = persist.tile([128, T], BF16)       # [local col, token]
    kTt = persist.tile([128, T], BF16)
    v_aug = persist.tile([128, NT, 144], FP8)  # [tok%128, tile, 2x(64 v + 1 + pad)]
    q_pre = persist.tile([128, NT, 128], BF16)  # [tok%128, tile, local col]
    k_pre = persist.tile([128, NT, 128], BF16)
    # stats cols: 0=sum_q, 1=sum_k, 2=ssq_q, 3=ssq_k
    stats = persist.tile([128, NCH, TCH, 4], F32)
    stats_all = persist.tile([128, NCH, TCH, 4], F32)
    o_un = persist.tile([128, 2 * B * HL, 1024], BF16)  # [dim(65), slot, qtok]
    onorm = persist.tile([128, T], BF16)
    siluo = persist.tile([128, T], BF16)
    scr = persist.tile([128, 128], BF16)

    ones_col = persist.tile([128, NT], F32)
    nc.vector.memset(ones_col, 1.0)
    nc.vector.tensor_copy(out=v_aug[:, :, 64:65], in_=ones_col)
    nc.vector.tensor_copy(out=v_aug[:, :, 136:137], in_=ones_col)

    dram = octx.enter_context(tc.tile_pool(name="dram", bufs=1, space="DRAM"))
    cc_in = [dram.tile([128, TCH * 4], F32, name=f"cc_in{c}")
             for c in range(NCH)]
    cc_out = [dram.tile([128, TCH * 4], F32, name=f"cc_out{c}",
                        addr_space="Shared")
              for c in range(NCH)]
    # start barrier: absorbs the cross-core launch skew before any real work,
    # so the stats AllReduces later see aligned peers
    bar_in = dram.tile([1, 4], F32, name="bar_in")
    bar_out = dram.tile([1, 4], F32, name="bar_out", addr_space="Shared")
    nc.gpsimd.collective_compute(
        "AllReduce", ALU.add,
        replica_groups=[list(range(n_cores))],
        ins=[bar_in.opt()], outs=[bar_out.opt()])

    ph1 = octx.enter_context(tc.tile_pool(name="ph1", bufs=3))
    ph1t = octx.enter_context(tc.tile_pool(name="ph1t", bufs=4))
    ph1s = octx.enter_context(tc.tile_pool(name="ph1s", bufs=4))
    ph2 = octx.enter_context(tc.tile_pool(name="ph2", bufs=1))
    ph3 = octx.enter_context(tc.tile_pool(name="ph3", bufs=8))

    pctx = ExitStack()           # phase-1 PSUM, closed before attention PSUM
    ph1q = pctx.enter_context(tc.tile_pool(name="ph1q", bufs=3, space="PSUM"))

    # ---------------- phase 1: x LN+silu, XBAR transpose, QKV ----------------
    # Split into a stats part (no PE dependency) and a compute part, emitted
    # one group ahead, so the DVE queue prefetches bn_stats instead of
    # head-of-line blocking on matmul-dependent evictions.
    def phase1_stats(g):
        xg = ph1.tile([128, GB, C], BF16, name="xg")
        nc.sync.dma_start(
            out=xg,
            in_=x[g * GB * 128:(g + 1) * GB * 128, :].rearrange(
                "(t p) c -> p t c", p=128))

        stats6 = ph1s.tile([128, GB, 2, 6], F32, name="stats6")
        for t in range(GB):
            for h2 in range(2):
                nc.vector.bn_stats(out=stats6[:, t, h2, :],
                                   in_=xg[:, t, h2 * 512:(h2 + 1) * 512])
        mv = ph1s.tile([128, GB, 2], F32, name="mv")
        for t in range(GB):
            nc.vector.bn_aggr(out=mv[:, t, :], in_=stats6[:, t, :, :])

        vpe = ph1s.tile([128, GB, 1], F32, name="vpe")
        nc.vector.tensor_scalar(out=vpe, in0=mv[:, :, 1:2], scalar1=EPS,
                                scalar2=None, op0=ALU.add)
        rstd = _quake_rsqrt(nc, ph1s, vpe, (128, GB, 1), iters=2, suffix="x")
        nmr = ph1s.tile([128, GB, 1], F32, name="nmr")
        nc.vector.tensor_tensor(out=nmr, in0=mv[:, :, 0:1], in1=rstd,
                                op=ALU.mult)
        nc.vector.tensor_scalar(out=nmr, in0=nmr, scalar1=-1.0,
                                scalar2=None, op0=ALU.mult)
        return xg, rstd, nmr

    def phase1_compute(g, pre):
        xg, rstd, nmr = pre
        for t in range(GB):
            tt = g * GB + t
            ch = tt // TCH
            ti = tt % TCH
            # silu(LN(x)) in place on ACT
            nc.scalar.activation(out=xg[:, t, :], in_=xg[:, t, :],
                                 func=AF.Silu,
                                 bias=nmr[:, t, :],
                                 scale=rstd[:, t, :])
            # x^T via one DMA XBAR op: [tok, 1024] -> [ch%128, ch//128, tok]
            xsT = ph1t.tile([128, KT, 128], BF16, name="xsT")
            nc.sync.dma_start_transpose(out=xsT, in_=xg[:, t, :])

            pqkv = ph1q.tile([128, QKVW], F32, name="pqkv")
            for kt in range(KT):
                nc.tensor.matmul(
                    pqkv,
                    lhsT=xsT[:, kt, :],
                    rhs=w_all_sb[:, kt, :],
                    start=(kt == 0), stop=(kt == KT - 1))

            # evictions (PSUM fp32 -> SBUF bf16/fp8) + bias
            nc.vector.scalar_tensor_tensor(
                out=q_pre[:, tt, :], in0=pqkv[:, 0:128], scalar=1.0,
                in1=b_sb[:, 0:128], op0=ALU.mult, op1=ALU.add)
            nc.vector.scalar_tensor_tensor(
                out=k_pre[:, tt, :], in0=pqkv[:, 128:256], scalar=1.0,
                in1=b_sb[:, 128:256], op0=ALU.mult, op1=ALU.add)
            nc.vector.scalar_tensor_tensor(
                out=v_aug[:, tt, :].rearrange("p (h e) -> p h e", e=72)[:, :, 0:64],
                in0=pqkv[:, 256:384].rearrange("p (h e) -> p h e", e=64),
                scalar=1.0,
                in1=b_sb[:, 256:384].rearrange("p (h e) -> p h e", e=64),
                op0=ALU.mult, op1=ALU.add)
            # q/k sums rode the matmul in the 2 extra weight columns
            # (+ sum-of-local-bias constant from b_sb)
            nc.vector.scalar_tensor_tensor(
                out=stats[:, ch, ti, 0:2], in0=pqkv[:, QKV:QKV + 2],
                scalar=1.0, in1=b_sb[:, QKV:QKV + 2],
                op0=ALU.mult, op1=ALU.add)
            # sums of squares on ACT
            nc.scalar.activation(
                out=scr, in_=q_pre[:, tt, :], func=AF.Square,
                accum_out=stats[:, ch, ti, 2:3])
            nc.scalar.activation(
                out=scr, in_=k_pre[:, tt, :], func=AF.Square,
                accum_out=stats[:, ch, ti, 3:4])

    def emit_allreduce(ch):
        nc.sync.dma_start(out=cc_in[ch],
                          in_=stats[:, ch].rearrange("p a b -> p (a b)"))
        nc.gpsimd.collective_compute(
            "AllReduce", ALU.add,
            replica_groups=[list(range(n_cores))],
            ins=[cc_in[ch].opt()], outs=[cc_out[ch].opt()])
        nc.sync.dma_start(
            out=stats_all[:, ch].rearrange("p a b -> p (a b)"),
            in_=cc_out[ch])

    # phase 2+3 for one chunk: full-inner LN stats -> normalize -> transpose
    def phase23_chunk(ch):
        qk_sn = []
        for which in range(2):  # 0 -> q, 1 -> k
            s_sum = stats_all[:, ch, :, which]
            s_ssq = stats_all[:, ch, :, 2 + which]
            m = ph2.tile([128, TCH], F32, name=f"m_{ch}_{which}")
            nc.vector.tensor_scalar(out=m, in0=s_sum, scalar1=1.0 / INNER,
                                    scalar2=None, op0=ALU.mult)
            msq = ph2.tile([128, TCH], F32, name=f"msq_{ch}_{which}")
            nc.vector.tensor_scalar(out=msq, in0=s_ssq, scalar1=1.0 / INNER,
                                    scalar2=None, op0=ALU.mult)
            tmp = ph2.tile([128, TCH], F32, name=f"tmp_{ch}_{which}")
            nc.vector.tensor_tensor(out=tmp, in0=m, in1=m, op=ALU.mult)
            nc.vector.tensor_tensor(out=tmp, in0=msq, in1=tmp, op=ALU.subtract)
            nc.vector.tensor_scalar(out=tmp, in0=tmp, scalar1=EPS,
                                    scalar2=None, op0=ALU.add)
            rstd = _quake_rsqrt(nc, ph2, tmp, (128, TCH),
                                suffix=f"_{ch}_{which}")
            nmr = ph2.tile([128, TCH], F32, name=f"nmr_{ch}_{which}")
            nc.vector.tensor_tensor(out=nmr, in0=m, in1=rstd, op=ALU.mult)
            nc.vector.tensor_scalar(out=nmr, in0=nmr, scalar1=-1.0,
                                    scalar2=None, op0=ALU.mult)
            qk_sn.append((m, rstd, nmr))

        for ti in range(TCH):
            tt = ch * TCH + ti
            (mq, rq, nq) = qk_sn[0]
            (mk, rk, nk) = qk_sn[1]
            qn = ph3.tile([128, 128], BF16, name="qn")
            nc.vector.tensor_scalar(
                out=qn, in0=q_pre[:, tt, :],
                scalar1=mq[:, ti:ti + 1], scalar2=rq[:, ti:ti + 1],
                op0=ALU.subtract, op1=ALU.mult)
            kn = ph3.tile([128, 128], BF16, name="kn")
            nc.scalar.activation(
                out=kn, in_=k_pre[:, tt, :], func=AF.Identity,
                bias=nk[:, ti:ti + 1], scale=rk[:, ti:ti + 1])
            nc.sync.dma_start_transpose(
                out=qT[:, tt * 128:(tt + 1) * 128], in_=qn)
            nc.sync.dma_start_transpose(
                out=kTt[:, tt * 128:(tt + 1) * 128], in_=kn)
            # gain/beta per tile (not per chunk) so attention S-matmuls can
            # stream in behind phase 3 instead of waiting on a chunk barrier
            lo, hi = tt * 128, (tt + 1) * 128
            nc.vector.tensor_scalar(
                out=qT[:, lo:hi], in0=qT[:, lo:hi],
                scalar1=gbe_sb[:, 0:1], scalar2=gbe_sb[:, 1:2],
                op0=ALU.mult, op1=ALU.add)
            nc.scalar.activation(
                out=kTt[:, lo:hi], in_=kTt[:, lo:hi], func=AF.Identity,
                bias=gbe_sb[:, 3:4], scale=gbe_sb[:, 2:3])

    # ---------------- phase 4: attention ----------------
    att = octx.enter_context(tc.tile_pool(name="att", bufs=3))
    dramsc = octx.enter_context(tc.tile_pool(name="dramsc", bufs=2,
                                             space="DRAM"))
    dnp = octx.enter_context(tc.tile_pool(name="dnp", bufs=2))
    actx = ExitStack()           # attention PSUM, closed before phase-5 PSUM

    NPAIR = KB // 2
    DR = mybir.MatmulPerfMode.DoubleRow

    def attention_bh(b, h, attp, attpo):
        for g in range(2):       # q-chunk group: tokens [g*1024, (g+1)*1024)
            slot = b * 4 + h * 2 + g

            def emit_pv(pO, eS2, pair):
                # fp8 DoubleRow PV: contract 256 keys (2 kb tiles) per matmul
                # at 0.5 cycles/row
                vt0 = b * KB + 2 * pair
                for qi in range(2):
                    nc.tensor.matmul(
                        pO[0:65, qi * 512:(qi + 1) * 512],
                        lhsT=v_aug[:, vt0:vt0 + 2, h * 72:h * 72 + 65],
                        rhs=eS2[:, :, qi * 512:(qi + 1) * 512],
                        start=(pair == 0), stop=(pair == NPAIR - 1),
                        perf_mode=DR)

            pO = attpo.tile([128, 1024], F32, name="pO", tag="pO")
            pend = None          # software pipeline: delay PV by one kb pair
            eS2 = None
            for kb in range(KB):
                pS = attp.tile([128, 1024], F32, name="pS", tag="pS")
                for qi in range(2):
                    q0 = b * N + g * 1024 + qi * 512
                    nc.tensor.matmul(
                        pS[:, qi * 512:(qi + 1) * 512],
                        lhsT=kTt[h * 64:(h + 1) * 64,
                                 b * N + kb * 128:b * N + (kb + 1) * 128],
                        rhs=qT[h * 64:(h + 1) * 64, q0:q0 + 512],
                        start=True, stop=True)
                if pend is not None and kb % 2 == 0:
                    emit_pv(pO, *pend)
                    pend = None
                if kb % 2 == 0:
                    eS2 = att.tile([128, 2, 1024], FP8, name="eS2")
                nc.scalar.activation(out=eS2[:, kb % 2, :], in_=pS,
                                     func=AF.Exp)
                if kb % 2 == 1:
                    pend = (eS2, kb // 2)
            emit_pv(pO, *pend)
            # evict unnormalized O + raw denominator row
            nc.vector.tensor_copy(out=o_un[0:65, slot, :], in_=pO[0:65, :])

    def denorm_batch(b):
        # batch b's denominators live in o_un[64, b*4:(b+1)*4, :]
        dn_dram = dramsc.tile([1, 4096], BF16, name="dn_dram")
        nc.sync.dma_start(
            out=dn_dram,
            in_=o_un[64:65, b * 4:(b + 1) * 4, :].rearrange(
                "p a t -> p (a t)"))
        dn_g = dnp.tile([128, 32], BF16, name="dn_g")
        nc.sync.dma_start(
            out=dn_g,
            in_=dn_dram[0:1, :].rearrange("o (p c) -> (o p) c", p=128))
        rdn = dnp.tile([128, 32], BF16, name="rdn")
        with nc.allow_low_precision(reason="softmax denom reciprocal, 2e-2 budget"):
            nc.vector.reciprocal(out=rdn, in_=dn_g)
        rdn_dram = dramsc.tile([1, 4096], BF16, name="rdn_dram")
        nc.sync.dma_start(
            out=rdn_dram[0:1, :].rearrange("o (p c) -> (o p) c", p=128),
            in_=rdn)
        dnb = dnp.tile([64, 4096], BF16, name="dnb")
        nc.sync.dma_start(out=dnb, in_=rdn_dram.to_broadcast([64, 4096]))
        for h in range(HL):
            for g in range(2):
                slot = b * 4 + h * 2 + g
                sg = h * 2 + g
                nc.vector.tensor_tensor(
                    out=onorm[h * 64:(h + 1) * 64,
                              b * N + g * 1024:b * N + (g + 1) * 1024],
                    in0=o_un[0:64, slot, :],
                    in1=dnb[:, sg * 1024:(sg + 1) * 1024],
                    op=ALU.mult)

    def silu_batch(b):
        nc.scalar.activation(out=siluo[:, b * N:(b + 1) * N],
                             in_=onorm[:, b * N:(b + 1) * N], func=AF.Silu)

    # ---------------- emission schedule ----------------
    pre = phase1_stats(0)
    for g in range(NG):
        nxt = phase1_stats(g + 1) if g + 1 < NG else None
        phase1_compute(g, pre)
        pre = nxt
        if g == 3:
            emit_allreduce(0)
    emit_allreduce(1)
    phase23_chunk(0)
    pctx.close()                 # free phase-1 PSUM banks
    attp = actx.enter_context(tc.tile_pool(name="attp", bufs=2, space="PSUM"))
    attpo = actx.enter_context(tc.tile_pool(name="attpo", bufs=2,
                                            space="PSUM"))
    attention_bh(0, 0, attp, attpo)
    phase23_chunk(1)
    attention_bh(0, 1, attp, attpo)
    denorm_batch(0)
    attention_bh(1, 0, attp, attpo)
    silu_batch(0)
    attention_bh(1, 1, attp, attpo)
    denorm_batch(1)
    silu_batch(1)
    actx.close()                 # free attention PSUM banks

    # ---------------- phase 5: output projection ----------------
    with tc.tile_pool(name="ph5", bufs=4) as ph5, \
         tc.tile_pool(name="ph5p", bufs=4, space="PSUM") as ph5p:
        for tk in range(T // 512):
            for ct in range(KT):
                po = ph5p.tile([128, 512], F32, name="po")
                nc.tensor.matmul(
                    po,
                    lhsT=w_o_sb[:, ct * 128:(ct + 1) * 128],
                    rhs=siluo[:, tk * 512:(tk + 1) * 512],
                    start=True, stop=True)
                ev = ph5.tile([128, 512], BF16, name="ev")
                if (tk * KT + ct) % 2 == 0:
                    nc.vector.tensor_copy(out=ev, in_=po)
                else:
                    nc.scalar.copy(out=ev, in_=po)
                nc.sync.dma_start(
                    out=out_t[ct * 128:(ct + 1) * 128,
                              tk * 512:(tk + 1) * 512],
                    in_=ev)

    octx.close()


def make_in_maps(inputs, n_tok_per_batch, n_cores=NCORES):
    """Slice full inputs into per-core input maps (head sharding)."""
    import ml_dtypes
    bf16 = ml_dtypes.bfloat16

    x = np.ascontiguousarray(np.asarray(inputs["x"], np.float32)
                             .reshape(B * n_tok_per_batch, C)).astype(bf16)
    w_q = np.asarray(inputs["w_q"], np.float32)
    w_k = np.asarray(inputs["w_k"], np.float32)
    w_v = np.asarray(inputs["w_v"], np.float32)
    b_q = np.asarray(inputs["b_q"], np.float32)
    b_k = np.asarray(inputs["b_k"], np.float32)
    b_v = np.asarray(inputs["b_v"], np.float32)
    g_q = np.asarray(inputs["g_q"], np.float32)
    be_q = np.asarray(inputs["be_q"], np.float32)
    g_k = np.asarray(inputs["g_k"], np.float32)
    be_k = np.asarray(inputs["be_k"], np.float32)
    w_o = np.asarray(inputs["w_o"], np.float32)

    scale = float(INNER) ** -0.5
    in_maps = []
    for c in range(n_cores):
        cols = slice(c * CL, (c + 1) * CL)
        wq_l = w_q[:, cols]
        wk_l = w_k[:, cols]
        wv_l = w_v[:, cols]
        w_all = np.ascontiguousarray(np.concatenate(
            [wq_l, wk_l, wv_l,
             wq_l.sum(axis=1, keepdims=True),
             wk_l.sum(axis=1, keepdims=True)], axis=1)).astype(bf16)
        b_all = np.ascontiguousarray(
            np.concatenate([b_q[cols], b_k[cols], b_v[cols],
                            [b_q[cols].sum()], [b_k[cols].sum()]])[None, :]
        ).astype(np.float32)
        gbe = np.ascontiguousarray(
            np.stack([g_q[cols] * scale, be_q[cols] * scale,
                      g_k[cols], be_k[cols]], axis=1))
        w_o_c = np.ascontiguousarray(w_o[cols, :]).astype(bf16)
        in_maps.append({
            "x": x, "w_all": w_all, "b_all": b_all,
            "gbe": gbe, "w_o_loc": w_o_c,
        })
    return in_maps


def combine_outputs(out_ts, inputs, n_tok_per_batch):
    b_o = np.asarray(inputs["b_o"], np.float32)
    acc = np.zeros(out_ts[0].shape, dtype=np.float32)
    for o in out_ts:
        acc += np.asarray(o, dtype=np.float32)
    out = acc.T + b_o[None, :]
    return out.reshape(B, n_tok_per_batch, C).astype(np.float32)


_NC_CACHE = {}


def kernel(**inputs):
    from concourse.bass_utils import run_bass_kernel_spmd

    n_tok = np.asarray(inputs["x"]).shape[1]
    if n_tok not in _NC_CACHE:
        _NC_CACHE[n_tok] = build_bass(n_tok)
    nc = _NC_CACHE[n_tok]
    in_maps = make_in_maps(inputs, n_tok)
    res = run_bass_kernel_spmd(nc, in_maps, core_ids=list(range(NCORES)))
    out_ts = [r["out_t"] for r in res.results]
    return combine_outputs(out_ts, inputs, n_tok)
